# revision 1
# baseline (speedup 1.0000x reference)
"""GPT2 attention + adapter kernel for 8 Trainium2 NeuronCores.

Sharding: core c -> (b = c//4, head-group g = c%4).  Each core owns 4 of the
16 heads for one batch element: data parallel over B, tensor parallel over
heads (c_attn columns / c_proj rows sharded with the heads; per-head gate
shards with heads).  The c_proj partial sums are combined on-device with a
ReduceScatter over each batch group of 4 cores; the host concatenates the 8
row-slices and adds c_proj_b.

Everything on-chip is computed in a transposed layout (feature dim on
partitions, sequence on the free axis) so no on-device transposes are needed:
  qT/kT  [64*2heads, S]  from  W^T @ x^T   (x^T supplied by host)
  scoresT[k, q] = kT.T-chunk x qT          (K=64, two heads row-packed)
  P^T = exp(scores/8) masked causally via gpsimd affine_select
  attn^T accum [65, q] = [v | 1].T @ P^T   (ones column -> softmax denom)
  y[q, D] = attnT.T-chunks @ Wproj-rows    (accumulated over head pairs)
"""

import sys

for _p in ("/opt/trn_rl_repo",):
    if _p not in sys.path:
        sys.path.insert(0, _p)

import numpy as np

# ---------------------------------------------------------------- constants
B, S, A, D, H, HD = 2, 2048, 64, 1024, 16, 64
N_CORES = 8
GPC = 4          # head-groups per batch
HPC = 4          # heads per core
SA = S + A       # 2112: hidden and adapter sequence concatenated
SCALE = 1.0 / 8.0
P = 128
NQC = S // 512   # 4 query chunks of 512
NST = S // P     # 16 s-tiles of 128
VW = HPC * 65    # v_sb row width per s-tile (4 heads x (64 dims + ones col))

_STATE: dict = {}


def _build_nc(reps=1, collective=True):
    import concourse.bacc as bacc
    import concourse.mybir as mybir
    import concourse.tile as tile
    from concourse.alu_op_type import AluOpType

    f32 = mybir.dt.float32
    f32r = mybir.dt.float32r
    AF = mybir.ActivationFunctionType

    nc = bacc.Bacc("TRN2", target_bir_lowering=False, debug=False,
                   num_devices=N_CORES)

    xa_t = nc.dram_tensor("xa_t", [8, P, SA], f32r, kind="ExternalInput").ap()
    w_qkv = nc.dram_tensor("w_qkv", [8, P, 768], f32r, kind="ExternalInput").ap()
    b_qkv = nc.dram_tensor("b_qkv", [4, P, 1], f32, kind="ExternalInput").ap()
    bv_eff = nc.dram_tensor("bv_eff", [4, 64, 1], f32, kind="ExternalInput").ap()
    gscale = nc.dram_tensor("gscale", [8, 1], f32, kind="ExternalInput").ap()
    w_proj = nc.dram_tensor("w_proj", [2, P, D], f32r, kind="ExternalInput").ap()
    selden = nc.dram_tensor("selden", [8, 65, 8], f32r, kind="ExternalInput").ap()
    selbc = nc.dram_tensor("selbc", [8, 8, 64], f32r, kind="ExternalInput").ap()
    ones_d = nc.dram_tensor("ones", [P, 64, 1], f32r, kind="ExternalInput").ap()
    out_ext = nc.dram_tensor("out", [512, D], f32, kind="ExternalOutput").ap()
    # dummy input whose shape encodes `reps` so each variant gets a distinct
    # HLO signature (the neuron compile cache ignores the embedded BIR)
    nc.dram_tensor("repsig", [reps, 1], f32, kind="ExternalInput")

    def r(ap):  # matmul-operand view (tiles are already f32r)
        return ap if ap.dtype == f32r else ap.bitcast(f32r)

    with tile.TileContext(nc) as tc, \
            nc.allow_low_precision(reason="fp32r matmul operand staging"):
        with (
            tc.tile_pool(name="persist", bufs=1) as pp,
            tc.tile_pool(name="dram", bufs=1, space="DRAM") as dp,
        ):
            # long-lived SBUF buffers
            qT2 = [pp.tile([P, S], f32r, tag=f"qT{i}", name=f"qT{i}") for i in range(2)]
            kT2 = [pp.tile([P, S], f32r, tag=f"kT{i}", name=f"kT{i}") for i in range(2)]
            kaT2 = [pp.tile([P, A], f32r, tag=f"kaT{i}", name=f"kaT{i}") for i in range(2)]
            v_sb = pp.tile([P, NST * VW], f32r, tag="v_sb", name="v_sb")
            va_sb = pp.tile([64, VW], f32r, tag="va_sb", name="va_sb")
            attn2 = [pp.tile([P, S], f32r, tag=f"attn{i}", name=f"attn{i}") for i in range(2)]
            wproj_sb = [pp.tile([P, D], f32r, tag=f"wp{i}", name=f"wp{i}") for i in range(2)]
            gsc_sb = pp.tile([8, 1], f32, tag="gsc", name="gsc")
            bvef_sb = [pp.tile([64, 1], f32, tag=f"bv{i}", name=f"bv{i}") for i in range(4)]
            sden_sb = [pp.tile([65, 8], f32r, tag=f"sd{j}", name=f"sd{j}") for j in range(8)]
            sbc_sb = [pp.tile([8, 64], f32r, tag=f"sb{j}", name=f"sb{j}") for j in range(8)]

            y_dram = dp.tile([S, D], f32, tag="ypart", name="ypart")
            rs_sh = dp.tile([512, D], f32, tag="rssh", name="rssh")

            for _rep in range(reps):
                # small-constant loads
                nc.sync.dma_start(out=gsc_sb[:], in_=gscale[:])
                for i in range(2):
                    nc.sync.dma_start(out=wproj_sb[i][:], in_=w_proj[i])
                for i in range(4):
                    nc.sync.dma_start(out=bvef_sb[i][:], in_=bv_eff[i])
                for j in range(8):
                    nc.sync.dma_start(out=sden_sb[j][:], in_=selden[j])
                    nc.sync.dma_start(out=sbc_sb[j][:], in_=selbc[j])

                # ---------------- stage 1: projections ----------------
                with (
                    tc.tile_pool(name="s1in", bufs=1) as s1p,
                    tc.tile_pool(name="s1ps", bufs=1, space="PSUM") as ps1,
                    tc.tile_pool(name="s1b", bufs=1) as s1b,
                ):
                    w_sb = [s1p.tile([P, 768], f32r, tag=f"w{k}", name=f"w{k}") for k in range(8)]
                    xa_sb = [s1p.tile([P, SA], f32r, tag=f"xa{k}", name=f"xa{k}") for k in range(8)]
                    for k in range(8):
                        nc.sync.dma_start(out=w_sb[k][:], in_=w_qkv[k])
                    for k in range(8):
                        nc.sync.dma_start(out=xa_sb[k][:], in_=xa_t[k])
                    bias_sb = [s1b.tile([P, 1], f32, tag=f"b{m}", name=f"b{m}") for m in range(4)]
                    for m in range(4):
                        nc.sync.dma_start(out=bias_sb[m][:], in_=b_qkv[m])

                    v_ones = v_sb.rearrange("p (t c) -> p t c", c=65)[:, :, 64:65]
                    nc.sync.dma_start(out=v_ones, in_=ones_d[:])
                    va_ones = va_sb.rearrange("p (t c) -> p t c", c=65)[:, :, 64:65]
                    nc.sync.dma_start(out=va_ones, in_=ones_d[0:64, 0:HPC, :])

                    # q/k in transposed layout: psum[cols128, s512] over 8 K-chunks
                    for m in range(4):          # q0 q1 k0 k1 (pairs of heads)
                        is_k = m >= 2
                        nch = 5 if is_k else 4  # k also projects the adapter rows
                        psl = [ps1.tile([P, 512], f32, tag=f"s1_{n}", name=f"s1_{n}") for n in range(nch)]
                        for kc in range(8):
                            lhs = r(w_sb[kc][:, m * P:(m + 1) * P])
                            for n in range(4):
                                nc.tensor.matmul(
                                    psl[n][:], lhs,
                                    r(xa_sb[kc][:, n * 512:(n + 1) * 512]),
                                    start=(kc == 0), stop=(kc == 7))
                            if is_k:
                                nc.tensor.matmul(
                                    psl[4][:, 0:A], lhs,
                                    r(xa_sb[kc][:, S:SA]),
                                    start=(kc == 0), stop=(kc == 7))
                        tgt = kT2[m - 2] if is_k else qT2[m]
                        for n in range(4):
                            nc.scalar.activation(
                                tgt[:, n * 512:(n + 1) * 512], psl[n][:],
                                AF.Identity, bias=bias_sb[m][:])
                        if is_k:
                            nc.scalar.activation(
                                kaT2[m - 2][:], psl[4][:, 0:A],
                                AF.Identity, bias=bias_sb[m][:])

                    # v in natural layout: psum[s128, vcols256] over 8 K-chunks
                    for st in range(NST):
                        psv = ps1.tile([P, 256], f32, tag="s1_v", name="s1_v", bufs=2)
                        for kc in range(8):
                            nc.tensor.matmul(
                                psv[:], r(xa_sb[kc][:, st * P:(st + 1) * P]),
                                r(w_sb[kc][:, 512:768]),
                                start=(kc == 0), stop=(kc == 7))
                        for h in range(HPC):
                            nc.vector.tensor_copy(
                                v_sb[:, st * VW + h * 65: st * VW + h * 65 + 64],
                                psv[:, h * 64:(h + 1) * 64])
                    # v_a
                    psva = ps1.tile([64, 256], f32, tag="s1_v", name="s1_va", bufs=2)
                    for kc in range(8):
                        nc.tensor.matmul(
                            psva[:], r(xa_sb[kc][:, S:SA]),
                            r(w_sb[kc][:, 512:768]),
                            start=(kc == 0), stop=(kc == 7))
                    for h in range(HPC):
                        nc.vector.tensor_copy(
                            va_sb[0:64, h * 65: h * 65 + 64],
                            psva[:, h * 64:(h + 1) * 64])

                # ---------------- stage 2: attention ----------------
                with (
                    tc.tile_pool(name="scps", bufs=3, space="PSUM") as scps,
                    tc.tile_pool(name="pvps", bufs=2, space="PSUM") as pvps,
                    tc.tile_pool(name="cpps", bufs=2, space="PSUM") as cpps,
                    tc.tile_pool(name="ptp", bufs=6) as ptp,
                    tc.tile_pool(name="osb", bufs=10) as osb,
                    tc.tile_pool(name="smal", bufs=4) as smal,
                ):
                    for qc in range(NQC):
                        qs = slice(qc * 512, (qc + 1) * 512)
                        o_main = [None] * HPC
                        o_adpt = [None] * HPC
                        for pair in range(2):
                            hA, hB = 2 * pair, 2 * pair + 1
                            # main attention, heads hA (rows 0:64) / hB (64:128)
                            pvA = pvps.tile([65, 512], f32, tag="pv", name="pv")
                            pvB = pvps.tile([65, 512], f32, tag="pv", name="pv")
                            nkb = 4 * qc + 4
                            for kb in range(nkb):
                                ssA = scps.tile([P, 512], f32, tag="sc", name="sc")
                                ssB = scps.tile([P, 512], f32, tag="sc", name="sc")
                                kslc = slice(kb * P, (kb + 1) * P)
                                nc.tensor.matmul(
                                    ssA[:], r(kT2[pair][0:64, kslc]),
                                    r(qT2[pair][0:64, qs]),
                                    start=True, stop=True, tile_position=(0, 0))
                                nc.tensor.matmul(
                                    ssB[:], r(kT2[pair][64:128, kslc]),
                                    r(qT2[pair][64:128, qs]),
                                    start=True, stop=True, tile_position=(64, 0))
                                ptA = ptp.tile([P, 512], f32r, tag="pt", name="pt")
                                ptB = ptp.tile([P, 512], f32r, tag="pt", name="pt")
                                nc.scalar.activation(ptA[:], ssA[:], AF.Exp,
                                                     bias=0.0, scale=SCALE)
                                nc.scalar.activation(ptB[:], ssB[:], AF.Exp,
                                                     bias=0.0, scale=SCALE)
                                if kb >= 4 * qc:  # diagonal block: causal mask
                                    base = qc * 512 - kb * P
                                    for pt in (ptA, ptB):
                                        nc.gpsimd.affine_select(
                                            out=pt[:], in_=pt[:],
                                            compare_op=AluOpType.is_ge,
                                            fill=0.0, base=base,
                                            pattern=[[1, 512]],
                                            channel_multiplier=-1)
                                st = kb
                                nc.tensor.matmul(
                                    pvA[:], r(v_sb[:, st * VW + hA * 65:
                                                   st * VW + hA * 65 + 65]),
                                    r(ptA[:]), start=(kb == 0), stop=(kb == nkb - 1))
                                nc.tensor.matmul(
                                    pvB[:], r(v_sb[:, st * VW + hB * 65:
                                                   st * VW + hB * 65 + 65]),
                                    r(ptB[:]), start=(kb == 0), stop=(kb == nkb - 1))
                            o_main[hA] = osb.tile([65, 512], f32r, tag="om", name="om")
                            o_main[hB] = osb.tile([65, 512], f32r, tag="om", name="om")
                            nc.vector.tensor_copy(o_main[hA][:], pvA[:])
                            nc.vector.tensor_copy(o_main[hB][:], pvB[:])

                            # adapter attention (no mask, 64 keys)
                            ssaA = scps.tile([64, 512], f32, tag="sca", name="sca", bufs=1)
                            ssaB = scps.tile([64, 512], f32, tag="sca", name="sca", bufs=1)
                            nc.tensor.matmul(
                                ssaA[:], r(kaT2[pair][0:64, :]),
                                r(qT2[pair][0:64, qs]),
                                start=True, stop=True, tile_position=(0, 0))
                            nc.tensor.matmul(
                                ssaB[:], r(kaT2[pair][64:128, :]),
                                r(qT2[pair][64:128, qs]),
                                start=True, stop=True, tile_position=(64, 0))
                            ptaA = ptp.tile([64, 512], f32r, tag="pta", name="pta")
                            ptaB = ptp.tile([64, 512], f32r, tag="pta", name="pta")
                            nc.scalar.activation(ptaA[:], ssaA[:], AF.Exp,
                                                 bias=0.0, scale=SCALE)
                            nc.scalar.activation(ptaB[:], ssaB[:], AF.Exp,
                                                 bias=0.0, scale=SCALE)
                            pvaA = pvps.tile([65, 512], f32, tag="pv", name="pv")
                            pvaB = pvps.tile([65, 512], f32, tag="pv", name="pv")
                            nc.tensor.matmul(
                                pvaA[:], r(va_sb[0:64, hA * 65: hA * 65 + 65]),
                                r(ptaA[:]), start=True, stop=True,
                                tile_position=(0, 0))
                            nc.tensor.matmul(
                                pvaB[:], r(va_sb[0:64, hB * 65: hB * 65 + 65]),
                                r(ptaB[:]), start=True, stop=True,
                                tile_position=(0, 0))
                            o_adpt[hA] = osb.tile([65, 512], f32r, tag="om", name="om")
                            o_adpt[hB] = osb.tile([65, 512], f32r, tag="om", name="om")
                            nc.vector.tensor_copy(o_adpt[hA][:], pvaA[:])
                            nc.vector.tensor_copy(o_adpt[hB][:], pvaB[:])

                        # gather the 8 softmax denominators (row 64 of each tile)
                        dps = pvps.tile([8, 512], f32, tag="pv", name="pv")
                        tiles = [o_main[h] for h in range(HPC)] + \
                                [o_adpt[h] for h in range(HPC)]
                        for j in range(8):
                            nc.tensor.matmul(dps[:], r(sden_sb[j][:]),
                                             r(tiles[j][:]),
                                             start=(j == 0), stop=(j == 7))
                        rec = smal.tile([8, 512], f32r, tag="rec", name="rec")
                        nc.vector.reciprocal(rec[:], dps[:])
                        nc.vector.tensor_scalar_mul(rec[:], rec[:], gsc_sb[:])

                        # per-head: broadcast recip rows, normalize, combine
                        for h in range(HPC):
                            pair, half = h // 2, h % 2
                            rbm = pvps.tile([64, 512], f32, tag="pv", name="pv")
                            rba = pvps.tile([64, 512], f32, tag="pv", name="pv")
                            nc.tensor.matmul(rbm[:], r(sbc_sb[h][:]), r(rec[:]),
                                             start=True, stop=True)
                            nc.tensor.matmul(rba[:], r(sbc_sb[4 + h][:]), r(rec[:]),
                                             start=True, stop=True)
                            t1 = smal.tile([64, 512], f32, tag="t1", name="t1")
                            t2 = smal.tile([64, 512], f32, tag="t2", name="t2")
                            nc.vector.tensor_tensor(t1[:], o_main[h].bitcast(f32)[0:64, :],
                                                    rbm[:], op=AluOpType.mult)
                            nc.vector.tensor_tensor(t2[:], o_adpt[h].bitcast(f32)[0:64, :],
                                                    rba[:], op=AluOpType.mult)
                            if half == 0:
                                nc.vector.scalar_tensor_tensor(
                                    attn2[pair][0:64, qs],
                                    t2[:], bvef_sb[h][:],
                                    t1[:], op0=AluOpType.add, op1=AluOpType.add)
                            else:
                                t3 = smal.tile([64, 512], f32r, tag="t3",
                                               name="t3")
                                nc.vector.scalar_tensor_tensor(
                                    t3[:], t2[:], bvef_sb[h][:],
                                    t1[:], op0=AluOpType.add, op1=AluOpType.add)
                                nc.sync.dma_start(out=attn2[pair][64:128, qs],
                                                  in_=t3[:])

                        # c_proj for this q-chunk
                        for qt in range(4 * qc, 4 * qc + 4):
                            for dc in range(2):
                                psy = cpps.tile([P, 512], f32, tag="y", name="y")
                                for pair in range(2):
                                    nc.tensor.matmul(
                                        psy[:],
                                        r(attn2[pair][:, qt * P:(qt + 1) * P]),
                                        r(wproj_sb[pair][:, dc * 512:(dc + 1) * 512]),
                                        start=(pair == 0), stop=(pair == 1))
                                ysb = smal.tile([P, 512], f32, tag="y", name="y")
                                nc.vector.tensor_copy(ysb[:], psy[:])
                                nc.sync.dma_start(
                                    out=y_dram[qt * P:(qt + 1) * P,
                                               dc * 512:(dc + 1) * 512],
                                    in_=ysb[:])

                # ---------------- stage 3: cross-core reduce ----------------
                if collective:
                    nc.gpsimd.collective_compute(
                        "ReduceScatter", AluOpType.add,
                        replica_groups=[[0, 1, 2, 3], [4, 5, 6, 7]],
                        ins=[y_dram.opt()], outs=[rs_sh.opt()])
                    nc.gpsimd.dma_start(out=out_ext[:], in_=rs_sh[:])
                else:
                    nc.gpsimd.dma_start(out=out_ext[:], in_=y_dram[0:512, :])

    nc.compile()
    return nc


def _make_in_maps(hidden_states, adapter, c_attn_w, c_attn_b, c_proj_w, gate):
    hidden_states = np.asarray(hidden_states, np.float32)
    adapter = np.asarray(adapter, np.float32)
    c_attn_w = np.asarray(c_attn_w, np.float32)
    c_attn_b = np.asarray(c_attn_b, np.float32)
    c_proj_w = np.asarray(c_proj_w, np.float32)
    gate = np.asarray(gate, np.float32)

    selden = np.zeros((8, 65, 8), np.float32)
    for j in range(8):
        selden[j, 64, j] = 1.0
    selbc = np.zeros((8, 8, 64), np.float32)
    for j in range(8):
        selbc[j, j, :] = 1.0

    in_maps = []
    for c in range(N_CORES):
        b, g = divmod(c, GPC)
        cs = slice(g * 256, (g + 1) * 256)
        w_qkv = np.concatenate(
            [c_attn_w[:, cs], c_attn_w[:, D:][:, cs], c_attn_w[:, 2 * D:][:, cs]],
            axis=1).reshape(8, P, 768)
        xa = np.concatenate([hidden_states[b], adapter[b]], axis=0)
        xa_t = np.ascontiguousarray(xa.T).reshape(8, P, SA)
        b_q = c_attn_b[cs]
        b_k = c_attn_b[D:][cs]
        b_v = c_attn_b[2 * D:][cs]
        b_qkv = np.concatenate([b_q, b_k]).reshape(4, P, 1)
        tg = np.tanh(gate[0, g * HPC:(g + 1) * HPC, 0, 0])
        gscale = np.concatenate([np.ones(4, np.float32),
                                 tg.astype(np.float32)]).reshape(8, 1)
        bv_eff = (b_v.reshape(HPC, HD) * (1.0 + tg)[:, None]).reshape(4, 64, 1)
        w_proj = np.ascontiguousarray(c_proj_w[cs, :]).reshape(2, P, D)
        in_maps.append({
            "xa_t": np.ascontiguousarray(xa_t),
            "w_qkv": np.ascontiguousarray(w_qkv),
            "b_qkv": np.ascontiguousarray(b_qkv),
            "bv_eff": np.ascontiguousarray(bv_eff.astype(np.float32)),
            "gscale": gscale.astype(np.float32),
            "w_proj": w_proj,
            "selden": selden,
            "selbc": selbc,
            "ones": np.ones((P, 64, 1), np.float32),
        })
    return in_maps


def _get_runner(reps=1):
    """Build + compile once; return f(in_maps) -> list[dict] (per-core)."""
    key = ("run", reps)
    if key in _STATE:
        return _STATE[key]

    import jax
    import jax.numpy as jnp  # noqa: F401
    from jax.experimental.shard_map import shard_map
    from jax.sharding import Mesh, PartitionSpec

    import concourse.mybir as mybir
    from concourse import bass2jax

    nc = _build_nc(reps)
    bass2jax.install_neuronx_cc_hook()

    partition_name = (nc.partition_id_tensor.name
                      if nc.partition_id_tensor else None)
    in_names, out_names, out_avals, zero_outs = [], [], [], []
    for alloc in nc.m.functions[0].allocations:
        if not isinstance(alloc, mybir.MemoryLocationSet):
            continue
        name = alloc.memorylocations[0].name
        if alloc.kind == "ExternalInput":
            if name != partition_name:
                in_names.append(name)
        elif alloc.kind == "ExternalOutput":
            shape = tuple(alloc.tensor_shape)
            dtype = mybir.dt.np(alloc.dtype)
            out_names.append(name)
            out_avals.append(jax.core.ShapedArray(shape, dtype))
            zero_outs.append(np.zeros(shape, dtype))
    in_shapes = {}
    for alloc in nc.m.functions[0].allocations:
        if isinstance(alloc, mybir.MemoryLocationSet) and alloc.kind == "ExternalInput":
            in_shapes[alloc.memorylocations[0].name] = (
                tuple(alloc.tensor_shape), mybir.dt.np(alloc.dtype))
    n_params = len(in_names)
    n_outs = len(out_avals)
    all_in_names = list(in_names) + list(out_names)
    if partition_name is not None:
        all_in_names.append(partition_name)
    donate = tuple(range(n_params, n_params + n_outs))

    def _body(*args):
        operands = list(args)
        if partition_name is not None:
            operands.append(bass2jax.partition_id_tensor())
        outs = bass2jax._bass_exec_p.bind(
            *operands,
            out_avals=tuple(out_avals),
            in_names=tuple(all_in_names),
            out_names=tuple(out_names),
            lowering_input_output_aliases=(),
            sim_require_finite=True,
            sim_require_nnan=True,
            nc=nc,
        )
        return tuple(outs)

    devices = jax.devices()[:N_CORES]
    mesh = Mesh(np.asarray(devices), ("core",))
    in_specs = (PartitionSpec("core"),) * (n_params + n_outs)
    out_specs = (PartitionSpec("core"),) * n_outs
    sharded = jax.jit(
        shard_map(_body, mesh=mesh, in_specs=in_specs, out_specs=out_specs,
                  check_rep=False),
        donate_argnums=donate, keep_unused=True)

    def run(in_maps, as_np=True):
        def get(c, n):
            if n in in_maps[c]:
                return np.asarray(in_maps[c][n])
            shape, dt_ = in_shapes[n]
            return np.zeros(shape, dt_)
        concat_in = [
            np.concatenate([get(c, n) for c in range(N_CORES)], axis=0)
            for n in in_names
        ]
        concat_zeros = [
            np.zeros((N_CORES * z.shape[0], *z.shape[1:]), z.dtype)
            for z in zero_outs
        ]
        out_arrs = sharded(*concat_in, *concat_zeros)
        if not as_np:
            return out_arrs
        return [
            {n: np.asarray(out_arrs[i]).reshape(N_CORES, *out_avals[i].shape)[c]
             for i, n in enumerate(out_names)}
            for c in range(N_CORES)
        ]

    run.in_names = in_names
    run.in_shapes = in_shapes
    run.out_names = out_names
    run.zero_outs = zero_outs
    run.sharded = sharded
    _STATE[key] = run
    return run


def kernel(hidden_states, adapter, c_attn_w, c_attn_b, c_proj_w, c_proj_b,
           gate):
    run = _get_runner()
    in_maps = _make_in_maps(hidden_states, adapter, c_attn_w, c_attn_b,
                            c_proj_w, gate)
    results = run(in_maps)
    out = np.empty((B, S, D), np.float32)
    for c in range(N_CORES):
        b, g = divmod(c, GPC)
        out[b, g * 512:(g + 1) * 512, :] = results[c]["out"]
    out += np.asarray(c_proj_b, np.float32)
    return out

